# revision 4
# baseline (speedup 1.0000x reference)
"""GridGenerator_Plus on 8 Trainium2 NeuronCores (Bass/Tile).

Pipeline:
  Stage 1 (device, batch-sharded, one NEFF): cross-attention transformer in a
  feature-major ("transposed") layout -> control points C^T (2 x 2048) per core.
  Host: batch-global pairwise-norm + bordered TPS solves in f64 via the Schur
  complement (K = hat matrix is shared across the batch).
  Stage 2 (device, batch-sharded, one NEFF): RBF lifting + P_hat @ T with the
  (b,j)-on-partitions layout; d2 via |P|^2 + |C|^2 - 2 P.C folded into matmuls.

All device matmuls are fp32 (4 cyc/row) for accuracy; the ill-conditioned TPS
solve amplifies upstream error ~1000x, so bf16/fp32r on the C path is unsafe.
Compiled NEFF + jitted PJRT executable are cached in module state so repeat
kernel() calls skip compilation.
"""
import numpy as np

B, L, D = 256, 1024, 64
H, DK = 4, 16
PY, PX = 4, 16
N = PY * PX                      # 64 fiducials
RH, RW = 32, 100
NG = RH * RW                     # 3200 grid points
NCORES = 8
BS = B // NCORES                 # 32 batches per core
GCH = [(i * 512, min(512, NG - i * 512)) for i in range((NG + 511) // 512)]

_RT = {}                         # cached runners / constants


# ----------------------------------------------------------------- host math
def _build_C64():
    gx, gy = np.meshgrid(np.linspace(-1, 1, PX), np.linspace(-1, 1, PY), indexing='ij')
    return np.stack([gx, gy], 2).reshape(-1, 2)          # (64, 2) f64


def _build_P64():
    gx = (np.arange(-RW, RW, 2, dtype=np.float64) + 1.0) / RW
    gy = (np.arange(-RH, RH, 2, dtype=np.float64) + 1.0) / RH
    mx, my = np.meshgrid(gx, gy, indexing='ij')
    return np.stack([mx, my], 2).reshape(-1, 2)          # (3200, 2) f64


def _consts_stage1(I):
    """Host-precomputed f32 constants for the stage-1 NEFF (f64 intermediate)."""
    W = {k: I[k].astype(np.float64) for k in I}
    f = np.float32
    ident = np.eye(128, dtype=f)
    wik = np.concatenate([W['W_in'] @ W['Wk'], (W['b_in'] @ W['Wk'] + W['bk'])[None]], 0).astype(f)
    wiv = np.concatenate([W['W_in'] @ W['Wv'], (W['b_in'] @ W['Wv'] + W['bv'])[None]], 0).astype(f)
    q = _build_C64() @ W['W_emb'] + W['b_emb']                       # (64, 64)
    qp = (q @ W['Wq'] + W['bq']) * 0.25                              # (64, 64) scaled
    qpbd = np.zeros((64, 256), np.float64)                           # block-diag qp^T
    for h in range(H):
        qpbd[16 * h:16 * h + 16, 64 * h:64 * h + 64] = qp[:, 16 * h:16 * h + 16].T
    qta = np.concatenate([q.T, np.ones((1, 64))], 0)                 # (65, 64) rhs
    qtb = np.concatenate([np.eye(64), I['bo'].astype(np.float64)[None]], 0)  # (65, 64) lhsT
    g1 = np.concatenate([np.diag(W['ln1_g']), W['ln1_b'][None]], 0)  # (65, 64)
    w1 = np.concatenate([np.diag(W['ln1_g']) @ W['W1'],
                         (W['ln1_b'] @ W['W1'] + W['b1'])[None]], 0)
    w2 = np.concatenate([W['W2'], W['b2'][None]], 0)
    wd = np.concatenate([np.diag(W['ln2_g']) @ W['W_down'],
                         (W['ln2_b'] @ W['W_down'] + W['b_down'])[None]], 0)  # (65, 2)
    return dict(ident=ident, wik=wik, wiv=wiv, qpbd=qpbd.astype(f), qta=qta.astype(f),
                qtb=qtb.astype(f), wo=I['Wo'].astype(f),
                g1=g1.astype(f), w1=w1.astype(f), w2=w2.astype(f), wd=wd.astype(f))


def _solve_T(C, bcp):
    """f64 TPS solve via Schur complement; K (=hat) is batch-global."""
    C = C.astype(np.float64)
    d = C[:, :, None, :] - C[:, None, :, :]
    sq = (d * d).sum((0, 3))
    r = np.sqrt(np.where(np.eye(N, dtype=bool), 1.0, sq))
    K = r * np.log(r)                                                # (64, 64) sym
    P = np.concatenate([np.ones((B, N, 1)), C], 2)                   # (B, 64, 3)
    Cp = bcp.astype(np.float64)                                      # (B, 64, 2)
    try:
        Ki = np.linalg.inv(K)
        M = np.einsum('jk,bkp->bjp', Ki, P, optimize=True)           # (B, 64, 3)
        KiCp = np.einsum('jk,bkp->bjp', Ki, Cp, optimize=True)       # (B, 64, 2)
        S = np.swapaxes(P, 1, 2) @ M                                 # (B, 3, 3)
        rhs = np.swapaxes(P, 1, 2) @ KiCp                            # (B, 3, 2)
        a = np.linalg.solve(S, rhs)                                  # (B, 3, 2)
        w = KiCp - M @ a                                             # (B, 64, 2)
    except np.linalg.LinAlgError:
        top = np.concatenate([P, np.broadcast_to(K, (B, N, N))], 2)
        mid = np.concatenate([np.zeros((B, 3, 3)), np.swapaxes(P, 1, 2)], 2)
        A = np.concatenate([top, mid], 1)
        Cpf = np.concatenate([Cp, np.zeros((B, 3, 2))], 1)
        T = np.linalg.solve(A, Cpf)
        return T[:, :3], T[:, 3:]
    return a, w                                                      # affine, rbf weights


# ------------------------------------------------------------- device builds
def _build_stage1():
    import concourse.bass as bass
    import concourse.tile as tile
    from concourse import bacc, mybir
    from concourse.mybir import ActivationFunctionType as AF, AluOpType as ALU
    f32 = mybir.dt.float32

    nc = bacc.Bacc("TRN2", target_bir_lowering=False, debug=False, num_devices=NCORES)
    iCf = nc.dram_tensor('cf', (BS, L, D), f32, kind="ExternalInput")
    cns = {n: nc.dram_tensor(n, s, f32, kind="ExternalInput") for n, s in
           [('ident', (128, 128)), ('wik', (65, 64)), ('wiv', (65, 64)),
            ('qpbd', (64, 256)), ('qta', (65, 64)), ('qtb', (65, 64)), ('wo', (64, 64)),
            ('g1', (65, 64)), ('w1', (65, 64)), ('w2', (65, 64)), ('wd', (65, 2))]}
    oCt = nc.dram_tensor('ct', (2, 2 * BS * N // 2), f32, kind="ExternalOutput")  # (2, 2048)

    with tile.TileContext(nc) as tc:
        from contextlib import ExitStack
        with ExitStack() as ex:
            sbc = ex.enter_context(tc.tile_pool(name="sbc", bufs=1))
            sb_cf = ex.enter_context(tc.tile_pool(name="sb_cf", bufs=3))
            sb_cfm = ex.enter_context(tc.tile_pool(name="sb_cfm", bufs=2))
            sb_kp = ex.enter_context(tc.tile_pool(name="sb_kp", bufs=2))
            sb_exp = ex.enter_context(tc.tile_pool(name="sb_exp", bufs=2))
            sb_att = ex.enter_context(tc.tile_pool(name="sb_att", bufs=2))
            sb_row = ex.enter_context(tc.tile_pool(name="sb_row", bufs=4))
            sb_x = ex.enter_context(tc.tile_pool(name="sb_x", bufs=1))
            ps_pst = ex.enter_context(tc.tile_pool(name="ps_pst", bufs=1, space="PSUM"))
            ps_kp = ex.enter_context(tc.tile_pool(name="ps_kp", bufs=1, space="PSUM"))
            ps_vp = ex.enter_context(tc.tile_pool(name="ps_vp", bufs=1, space="PSUM"))
            ps_sc = ex.enter_context(tc.tile_pool(name="ps_sc", bufs=2, space="PSUM"))
            ps_av = ex.enter_context(tc.tile_pool(name="ps_av", bufs=1, space="PSUM"))
            ps_o = ex.enter_context(tc.tile_pool(name="ps_o", bufs=1, space="PSUM"))
            ps_stat = ex.enter_context(tc.tile_pool(name="ps_stat", bufs=2, space="PSUM"))

            ct = {n: sbc.tile(list(t.shape), f32, tag='c_' + n) for n, t in cns.items()}
            for n in ct:
                nc.sync.dma_start(ct[n][:], cns[n].ap())
            oneinv = sbc.tile([64, 1], f32)
            nc.vector.memset(oneinv[:], 1.0 / 64.0)
            ones1 = sbc.tile([1, 64], f32)
            nc.vector.memset(ones1[:], 1.0)

            # persistent transposed activations
            x1t = sbc.tile([64, 64 * BS], f32)                     # q + o (pre-LN1)
            vpo = sbc.tile([128, 8 * 65], f32)                     # [vp(64)|1] per tok-tile
            for t in range(8):
                nc.vector.memset(vpo[:, t * 65 + 64:t * 65 + 65], 1.0)

            for b in range(BS):
                cfs = sb_cf.tile([128, 512], f32)                  # 8 tok-tiles, 64 cols each
                nc.sync.dma_start(cfs[:], iCf.ap()[b].rearrange("(t p) d -> p (t d)", p=128))
                cfm = sb_cfm.tile([65, 1024], f32)                 # feature-major + ones row
                nc.vector.memset(cfm[64:65, :], 1.0)
                for t in range(8):
                    pt = ps_pst.tile([64, 128], f32, tag="pst")
                    nc.tensor.transpose(pt[:], cfs[:, t * 64:(t + 1) * 64], ct['ident'][:])
                    nc.vector.tensor_copy(cfm[0:64, t * 128:(t + 1) * 128], pt[:])
                kpp = ps_kp.tile([64, 1024], f32, tag="pskp")
                for hh in range(2):
                    nc.tensor.matmul(kpp[:, hh * 512:(hh + 1) * 512], ct['wik'][:],
                                     cfm[:, hh * 512:(hh + 1) * 512], start=True, stop=True)
                kp = sb_kp.tile([64, 1024], f32)
                nc.scalar.activation(kp[:], kpp[:], AF.Copy)
                vpp = ps_vp.tile([128, 512], f32)
                for t in range(8):
                    nc.tensor.matmul(vpp[:, t * 64:(t + 1) * 64],
                                     cfm[:, t * 128:(t + 1) * 128], ct['wiv'][:],
                                     start=True, stop=True)
                nc.vector.tensor_copy(
                    vpo[:].rearrange("p (t c) -> p t c", c=65)[:, :, 0:64],
                    vpp[:].rearrange("p (t c) -> p t c", c=64))

                expt = sb_exp.tile([128, 2048], f32)
                for hf in range(4):                                # 2 tok-tiles per chunk
                    scp = ps_sc.tile([128, 512], f32, tag="sc")
                    for tt in range(2):
                        t = hf * 2 + tt
                        nc.tensor.matmul(scp[:, tt * 256:(tt + 1) * 256],
                                         kp[:, t * 128:(t + 1) * 128], ct['qpbd'][:],
                                         start=True, stop=True)
                    nc.scalar.activation(expt[:, hf * 512:(hf + 1) * 512], scp[:], AF.Exp)
                avp = ps_av.tile([65, 256], f32, tag="av")
                for t in range(8):
                    nc.tensor.matmul(avp[:], vpo[:, t * 65:(t + 1) * 65],
                                     expt[:, t * 256:(t + 1) * 256],
                                     start=(t == 0), stop=(t == 7))
                # reciprocal of denominators + one Newton step
                r0 = sb_row.tile([1, 256], f32, tag="r0")
                nc.vector.reciprocal(r0[:], avp[64:65, :])
                n1 = sb_row.tile([1, 256], f32, tag="n1")
                nc.vector.tensor_mul(n1[:], avp[64:65, :], r0[:])
                nc.vector.tensor_scalar(n1[:], n1[:], -1.0, 2.0, ALU.mult, ALU.add)
                nc.vector.tensor_mul(r0[:], r0[:], n1[:])
                dbc = ps_sc.tile([64, 256], f32, tag="sc")         # bcast recip to 64 rows
                nc.tensor.matmul(dbc[:], ones1[:], r0[:], start=True, stop=True)
                att = sb_att.tile([64, 256], f32)
                nc.vector.tensor_mul(att[:], avp[0:64, :], dbc[:])
                op = ps_o.tile([64, 64], f32, tag="o")
                nc.tensor.matmul(op[:], ct['qtb'][:], ct['qta'][:], start=True, stop=False)
                for h in range(4):
                    nc.tensor.matmul(op[:], ct['wo'][16 * h:16 * h + 16, :],
                                     att[16 * h:16 * h + 16, 64 * h:64 * h + 64],
                                     start=False, stop=(h == 3))
                nc.scalar.activation(x1t[:, b * 64:(b + 1) * 64], op[:], AF.Copy)

            # ---- batch-level tail: LN1 -> FFN(+residual) -> LN2 -> W_down ----
            def layer_norm(src, dst_aug):
                """dst_aug[0:64] = (src - mean_d) * rstd ; per-column stats via matmul."""
                for c in range(4):
                    cs = slice(c * 512, (c + 1) * 512)
                    xx = sb_exp.tile([64, 512], f32, tag="xx")
                    nc.scalar.activation(xx[:], src[:, cs], AF.Square)
                    mp = ps_stat.tile([1, 512], f32, tag="stat")
                    nc.tensor.matmul(mp[:], oneinv[:], src[:, cs], start=True, stop=True)
                    sp = ps_stat.tile([1, 512], f32, tag="stat")
                    nc.tensor.matmul(sp[:], oneinv[:], xx[:], start=True, stop=True)
                    t1 = sb_row.tile([1, 512], f32, tag="t1")
                    nc.vector.tensor_mul(t1[:], mp[:], mp[:])
                    nc.vector.tensor_sub(t1[:], sp[:], t1[:])       # var
                    st = sb_row.tile([1, 512], f32, tag="st")
                    nc.scalar.activation(st[:], t1[:], AF.Sqrt, bias=1e-5)
                    r0 = sb_row.tile([1, 512], f32, tag="r0b")
                    nc.vector.reciprocal(r0[:], st[:])
                    n1 = sb_row.tile([1, 512], f32, tag="n1b")
                    nc.vector.tensor_mul(n1[:], st[:], r0[:])
                    nc.vector.tensor_scalar(n1[:], n1[:], -1.0, 2.0, ALU.mult, ALU.add)
                    nc.vector.tensor_mul(r0[:], r0[:], n1[:])       # rstd
                    mg = sb_row.tile([1, 512], f32, tag="mg")
                    nc.vector.tensor_mul(mg[:], mp[:], r0[:])       # mean*rstd
                    ab = ps_sc.tile([64, 512], f32, tag="sc")
                    nc.tensor.matmul(ab[:], ones1[:], r0[:], start=True, stop=True)
                    bb = ps_sc.tile([64, 512], f32, tag="sc")
                    nc.tensor.matmul(bb[:], ones1[:], mg[:], start=True, stop=True)
                    nc.vector.tensor_mul(dst_aug[0:64, cs], src[:, cs], ab[:])
                    nc.vector.tensor_sub(dst_aug[0:64, cs], dst_aug[0:64, cs], bb[:])

            xh1 = sb_x.tile([65, 2048], f32)
            nc.vector.memset(xh1[64:65, :], 1.0)
            layer_norm(x1t, xh1)
            x2p = sb_x.tile([64, 2048], f32)
            for c in range(4):
                cs = slice(c * 512, (c + 1) * 512)
                hp = ps_sc.tile([64, 512], f32, tag="sc")
                nc.tensor.matmul(hp[:], ct['w1'][:], xh1[:, cs], start=True, stop=True)
                ht = sb_exp.tile([65, 512], f32, tag="ht")
                nc.vector.memset(ht[64:65, :], 1.0)
                nc.scalar.activation(ht[0:64, :], hp[:], AF.Relu)
                fp = ps_sc.tile([64, 512], f32, tag="sc")
                nc.tensor.matmul(fp[:], ct['w2'][:], ht[:], start=True, stop=False)
                nc.tensor.matmul(fp[:], ct['g1'][:], xh1[:, cs], start=False, stop=True)
                nc.scalar.activation(x2p[:, cs], fp[:], AF.Copy)
            xh2 = sb_x.tile([65, 2048], f32)
            nc.vector.memset(xh2[64:65, :], 1.0)
            layer_norm(x2p, xh2)
            ctile = sb_x.tile([2, 2048], f32)
            for c in range(4):
                cs = slice(c * 512, (c + 1) * 512)
                cp = ps_o.tile([2, 512], f32, tag="o")
                nc.tensor.matmul(cp[:], ct['wd'][:], xh2[:, cs], start=True, stop=True)
                nc.vector.tensor_copy(ctile[:, cs], cp[:])
            nc.sync.dma_start(oCt.ap(), ctile[:])
    nc.compile()
    return nc


def _build_stage2():
    import concourse.bass as bass
    import concourse.tile as tile
    from concourse import bacc, mybir
    from concourse.mybir import ActivationFunctionType as AF, AluOpType as ALU
    f32 = mybir.dt.float32

    nc = bacc.Bacc("TRN2", target_bir_lowering=False, debug=False, num_devices=NCORES)
    iL = nc.dram_tensor('lhs3', (3, 2048), f32, kind="ExternalInput")    # [-2Cx; -2Cy; 1]
    iCc = nc.dram_tensor('ccc', (128, 16), f32, kind="ExternalInput")    # |C|^2 per (pair)
    iTr = nc.dram_tensor('trbf', (128, 64), f32, kind="ExternalInput")   # block-diag 0.5*T_w
    iTa = nc.dram_tensor('taff', (3, 64), f32, kind="ExternalInput")     # T_affine
    iPg = nc.dram_tensor('paug', (3, NG), f32, kind="ExternalInput")     # [Px; Py; |P|^2]
    iPa = nc.dram_tensor('paffb', (3, NG), f32, kind="ExternalInput")    # [1; Px; Py]
    oY = nc.dram_tensor('yout', (64, NG), f32, kind="ExternalOutput")

    with tile.TileContext(nc) as tc:
        from contextlib import ExitStack
        with ExitStack() as ex:
            sbc = ex.enter_context(tc.tile_pool(name="sbc", bufs=1))
            sbw = ex.enter_context(tc.tile_pool(name="sbw", bufs=3))
            ps_d = ex.enter_context(tc.tile_pool(name="ps_d", bufs=3, space="PSUM"))
            ps_y = ex.enter_context(tc.tile_pool(name="ps_y", bufs=2, space="PSUM"))

            tL = sbc.tile([3, 2048], f32);  nc.sync.dma_start(tL[:], iL.ap())
            tCc = sbc.tile([128, 16], f32); nc.sync.dma_start(tCc[:], iCc.ap())
            tTr = sbc.tile([128, 64], f32); nc.sync.dma_start(tTr[:], iTr.ap())
            tTa = sbc.tile([3, 64], f32);   nc.sync.dma_start(tTa[:], iTa.ap())
            tPg = sbc.tile([3, NG], f32);   nc.sync.dma_start(tPg[:], iPg.ap())
            tPa = sbc.tile([3, NG], f32);   nc.sync.dma_start(tPa[:], iPa.ap())
            tY = sbc.tile([64, NG], f32)

            for p in range(16):
                for g0, gn in GCH:
                    dp = ps_d.tile([128, 512], f32, tag="d2")
                    nc.tensor.matmul(dp[:, 0:gn], tL[:, p * 128:(p + 1) * 128],
                                     tPg[:, g0:g0 + gn], start=True, stop=True)
                    d2 = sbw.tile([128, 512], f32, tag="d2s")
                    nc.vector.tensor_scalar(d2[:, 0:gn], dp[:, 0:gn],
                                            tCc[:, p:p + 1], 1e-20, ALU.add, ALU.max)
                    lg = sbw.tile([128, 512], f32, tag="lg")
                    nc.scalar.activation(lg[:, 0:gn], d2[:, 0:gn], AF.Ln)
                    nc.vector.tensor_mul(d2[:, 0:gn], d2[:, 0:gn], lg[:, 0:gn])
                    yp = ps_y.tile([4, 512], f32, tag="y")
                    nc.tensor.matmul(yp[:, 0:gn], tTr[:, 4 * p:4 * p + 4],
                                     d2[:, 0:gn], start=True, stop=False)
                    nc.tensor.matmul(yp[:, 0:gn], tTa[:, 4 * p:4 * p + 4],
                                     tPa[:, g0:g0 + gn], start=False, stop=True)
                    nc.scalar.activation(tY[4 * p:4 * p + 4, g0:g0 + gn], yp[:, 0:gn], AF.Copy)
            nc.sync.dma_start(oY.ap(), tY[:])
    nc.compile()
    return nc


# ------------------------------------------------------------- cached runner
def _make_runner(nc, n_cores):
    """Build ONE jitted shard_map executable for nc (mirrors
    bass2jax.run_bass_via_pjrt but caches the jit across calls)."""
    import jax
    import numpy as _np
    from jax.sharding import Mesh, PartitionSpec
    from jax.experimental.shard_map import shard_map
    from concourse import bass2jax, mybir
    bass2jax.install_neuronx_cc_hook()

    assert nc.dbg_addr is None or not nc.dbg_callbacks
    extra = {}
    if nc.dbg_addr is not None:
        extra[nc.dbg_addr.name] = _np.zeros((1, 2), _np.uint32)
    partition_name = nc.partition_id_tensor.name if nc.partition_id_tensor else None

    in_names, out_names, out_avals, zero_shapes = [], [], [], []
    for alloc in nc.m.functions[0].allocations:
        if not isinstance(alloc, mybir.MemoryLocationSet):
            continue
        name = alloc.memorylocations[0].name
        if alloc.kind == "ExternalInput":
            if name != partition_name:
                in_names.append(name)
        elif alloc.kind == "ExternalOutput":
            out_names.append(name)
            shape = tuple(alloc.tensor_shape)
            dtype = mybir.dt.np(alloc.dtype)
            out_avals.append(jax.core.ShapedArray(shape, dtype))
            zero_shapes.append((shape, dtype))
    n_params = len(in_names)
    n_outs = len(out_avals)
    all_names = list(in_names) + list(out_names)
    if partition_name is not None:
        all_names.append(partition_name)
    donate = tuple(range(n_params, n_params + n_outs))

    def _body(*args):
        operands = list(args)
        if partition_name is not None:
            operands.append(bass2jax.partition_id_tensor())
        outs = bass2jax._bass_exec_p.bind(
            *operands, out_avals=tuple(out_avals), in_names=tuple(all_names),
            out_names=tuple(out_names), lowering_input_output_aliases=(),
            sim_require_finite=True, sim_require_nnan=True, nc=nc)
        return tuple(outs)

    devices = jax.devices()[:n_cores]
    mesh = Mesh(np.asarray(devices), ("core",))
    in_specs = (PartitionSpec("core"),) * (n_params + n_outs)
    out_specs = (PartitionSpec("core"),) * n_outs
    sharded = jax.jit(shard_map(_body, mesh=mesh, in_specs=in_specs,
                                out_specs=out_specs, check_rep=False),
                      donate_argnums=donate, keep_unused=True)

    def run(in_maps):
        per_core = [[_np.ascontiguousarray(m[nm]) for nm in in_names] for m in
                    ({**m, **extra} for m in in_maps)]
        concat_in = [_np.concatenate([per_core[c][i] for c in range(n_cores)], axis=0)
                     for i in range(n_params)]
        concat_zeros = [_np.zeros((n_cores * s[0], *s[1:]), dt) for s, dt in zero_shapes]
        out_arrs = sharded(*concat_in, *concat_zeros)
        return [{nm: _np.asarray(out_arrs[i]).reshape(n_cores, *out_avals[i].shape)[c]
                 for i, nm in enumerate(out_names)} for c in range(n_cores)]

    return run


def _get_runtime():
    if 'r1' not in _RT:
        nc1 = _build_stage1()
        _RT['r1'] = _make_runner(nc1, NCORES)
        nc2 = _build_stage2()
        _RT['r2'] = _make_runner(nc2, NCORES)
    return _RT['r1'], _RT['r2']


# ------------------------------------------------------------- numpy fallback
def _kernel_numpy(I):
    f = np.float32
    g = {k: v.astype(f) for k, v in I.items()}
    Cf = g['C_feat'].reshape(B * L, D)
    cs = _consts_stage1(I)
    kp = (Cf @ cs['wik'][:64] + cs['wik'][64]).reshape(B, L, H, DK)
    vp = (Cf @ cs['wiv'][:64] + cs['wiv'][64]).reshape(B, L, H, DK)
    qp4 = (cs['qpbd'].T.reshape(4, 64, 64).transpose(1, 0, 2))
    qp = np.zeros((N, H, DK), f)
    for h in range(H):
        qp[:, h, :] = cs['qpbd'][16 * h:16 * h + 16, 64 * h:64 * h + 64].T
    kp_t = kp.transpose(0, 2, 1, 3); vp_t = vp.transpose(0, 2, 1, 3)
    sc = qp.transpose(1, 0, 2)[None] @ kp_t.transpose(0, 1, 3, 2)
    e = np.exp(sc)
    num = e @ vp_t
    den = e.sum(-1)
    o = (num / den[..., None]).transpose(0, 2, 1, 3).reshape(B, N, D) @ g['Wo'] + g['bo']
    q = cs['qta'][:64].T
    def lnf(x, gg, bb):
        m = x.mean(-1, keepdims=True)
        v = ((x - m) ** 2).mean(-1, keepdims=True)
        return (x - m) / np.sqrt(v + f(1e-5)) * gg + bb
    x = lnf(q[None] + o, g['ln1_g'], g['ln1_b'])
    x = lnf(x + np.maximum(x @ g['W1'] + g['b1'], 0) @ g['W2'] + g['b2'],
            g['ln2_g'], g['ln2_b'])
    C = (x @ g['W_down'] + g['b_down']).astype(f)
    a, w = _solve_T(C, I['batch_C_prime'])
    P = _build_P64().astype(f)
    pp = (P * P).sum(1)
    cc = (C * C).sum(2)
    pc = C @ P.T
    d2 = np.maximum(pp[None, None] + cc[:, :, None] - 2 * pc, f(1e-20))
    rbf = (0.5 * d2 * np.log(d2)).transpose(0, 2, 1)
    aff = np.concatenate([np.ones((NG, 1), f), P], 1)
    y = aff @ a.astype(f) + rbf @ w.astype(f)
    return y.astype(f)


# ----------------------------------------------------------------- kernel()
def kernel(**inputs):
    I = {k: np.asarray(v) for k, v in inputs.items()}
    try:
        return _kernel_device(I)
    except Exception:
        import traceback
        traceback.print_exc()
        return _kernel_numpy(I)


def _kernel_device(I):
    r1, r2 = _get_runtime()
    cs = _consts_stage1(I)
    Cf = np.ascontiguousarray(I['C_feat'].astype(np.float32))
    in1 = [{'cf': Cf[c * BS:(c + 1) * BS], **cs} for c in range(NCORES)]
    out1 = r1(in1)
    # ct: (2, 2048) per core, cols = (b_local, n) -> C (B, 64, 2)
    C = np.concatenate([o['ct'].reshape(2, BS, N).transpose(1, 2, 0) for o in out1], 0)
    C = np.ascontiguousarray(C, np.float32)

    a, w = _solve_T(C, I['batch_C_prime'])               # f64 (B,3,2), (B,64,2)

    P = _build_P64()
    paug = np.stack([P[:, 0], P[:, 1], (P * P).sum(1)], 0).astype(np.float32)
    paffb = np.stack([np.ones(NG), P[:, 0], P[:, 1]], 0).astype(np.float32)
    in2 = []
    for c in range(NCORES):
        Cc = C[c * BS:(c + 1) * BS].astype(np.float64)   # (32, 64, 2)
        lhs3 = np.empty((3, 2048), np.float32)
        lhs3[0] = (-2.0 * Cc[:, :, 0]).reshape(-1)
        lhs3[1] = (-2.0 * Cc[:, :, 1]).reshape(-1)
        lhs3[2] = 1.0
        ccc = (Cc * Cc).sum(2).reshape(16, 128).T.astype(np.float32)  # [128,(pair)]
        trbf = np.zeros((128, 64), np.float32)
        taff = np.zeros((3, 64), np.float32)
        for p in range(16):
            for s in range(2):                            # 2 batches per pair
                bg = c * BS + p * 2 + s
                trbf[64 * s:64 * s + 64, 4 * p + 2 * s:4 * p + 2 * s + 2] = \
                    0.5 * w[bg].astype(np.float32)
                taff[:, 4 * p + 2 * s:4 * p + 2 * s + 2] = a[bg].astype(np.float32)
        in2.append({'lhs3': lhs3, 'ccc': np.ascontiguousarray(ccc),
                    'trbf': trbf, 'taff': taff, 'paug': paug, 'paffb': paffb})
    out2 = r2(in2)
    Y = np.stack([o['yout'] for o in out2], 0)            # (8, 64, 3200)
    y = np.ascontiguousarray(
        Y.reshape(NCORES, 16, 2, 2, NG).transpose(0, 1, 2, 4, 3)).reshape(B, NG, 2)
    return y.astype(np.float32)


if __name__ == '__main__':
    import time
    rng = np.random.default_rng(0)
    fake = {'batch_C_prime': (rng.standard_normal((B, N, 2)) * 0.5).astype(np.float32),
            'C_feat': rng.standard_normal((B, L, D)).astype(np.float32)}
    for k, shape in [('W_in', (D, D)), ('W_emb', (2, D)), ('W_down', (D, 2)),
                     ('Wq', (D, D)), ('Wk', (D, D)), ('Wv', (D, D)), ('Wo', (D, D)),
                     ('W1', (D, D)), ('W2', (D, D))]:
        fake[k] = (rng.standard_normal(shape) / np.sqrt(shape[0])).astype(np.float32)
    for k, n in [('b_in', D), ('b_emb', D), ('b_down', 2), ('bq', D), ('bk', D),
                 ('bv', D), ('bo', D), ('b1', D), ('b2', D), ('ln1_b', D), ('ln2_b', D)]:
        fake[k] = np.zeros(n, np.float32)
    fake['ln1_g'] = np.ones(D, np.float32)
    fake['ln2_g'] = np.ones(D, np.float32)
    t0 = time.time()
    y = kernel(**fake)
    print('cold %.2fs out %s' % (time.time() - t0, y.shape))
    t0 = time.time()
    y = kernel(**fake)
    print('warm %.2fs' % (time.time() - t0))
